# revision 11
# baseline (speedup 1.0000x reference)
"""BuzzLoss Trainium2 kernel — fused custom-DVE op + adaptive tail truncation.

Math (telescoped form of the reference):
    excl[t] = prod_{s<t} (1 - conf[s])          (exclusive cumprod)
    score_b = sum_t excl[b,t] * da[b,t]
    da[b,0] = acc[b,0];  da[b,t] = acc[b,t] - acc[b,t-1]
    out = -mean_b score_b

With k = t-1 this is  score_b = acc[b,0] + sum_{k>=0} incl[k] * da[k+1]
where incl[k] = prod_{j<=k} nb[j], nb = 1 - conf.  Per 128-row tile the whole
inner sum is ONE custom-DVE instruction:

    Spec(body=scan(MULT, Src0) * Src1, accum=add)
      accum_out[p] = sum_k (prod_{j<=k} Src0[p,j]) * Src1[p,k]

The scan combine uses same-stage CURR_ALU_OUT feedback (no pipeline bubble),
so the op streams at 1 elem/cycle/lane with an fp32 recurrence state — vs the
stock tensor_tensor_scan (half rate) + separate multiply-accumulate pass.

Adaptive tail truncation: incl[k] decays geometrically (each factor <= 1),
so once it provably drops below 2^-30 every remaining term of the row is
|incl*da| <= 2^-30, and the dropped tail is bounded by (T-Tcut)*2^-30 < 1e-6
absolute (rel ~2e-6) — four orders below the 2e-2 budget.  The host computes
log2-cumsums of the ACTUAL (fp8-rounded) nb it ships and picks the smallest
Tcut in {48, 64, 128, 256, 512, T} whose worst row across the batch passes
the 2^-30 bound; the device streams only Tcut columns.  This is not a
distributional assumption: adversarial inputs simply select a larger Tcut
(up to full length) and stay exact; typical uniform-confidence data passes
at Tcut=48 (max row log2 incl[47] = -35).

Host prep is otherwise dtype/layout only (all reduction work on device):
    nb = fp8_e4m3(1 - conf), dash = int8(acc[:,1:] - acc[:,:-1])
both packed per core into ONE u8 tensor [128, 2*NTILES*Tcut] (partition p
holds tile j's row j*128+p; nb bytes in the first NTILES sub-blocks, dash in
the second) so each core's entire input is a single dense DMA, bitcast back
to fp8/int8 on-chip.
fp8 keeps ~3-bit relative precision on nb (floating format, so the small nb
near conf~1 that drive the decay stay accurate); end-to-end rel err 7e-05.
dash in {-1,0,1} is exact in int8; the DVE converts both dtypes on read.

The t=0 boundary term (= acc[b,0]) and the final mean are host-side, as is
the cross-core reduction (pure data parallel, batch 8192 = 8 x 1024 rows).
The 8 row-tiles per core run as ONE DVE instruction: the in0/in1 APs are
[128, 8, Tcut] and a hand-patched third uop reseeds the scan state at each
SUB_DIM_DONE boundary (keeping the accum register), so per-instruction
dispatch overhead is paid once instead of 8x.  accum_out[p] sums the 8 rows
partition p carries — reduce_partials sums everything anyway.
Steady state at Tcut=48: DVE ~(384+66+7) cyc @ 0.96 GHz ~ 0.48 us/core,
one 96 KiB DMA/core (768 B/partition) — vs 32 us for the fp32 scan+stt
baseline, 9.2 us full-length fused-op, 1.7 us Tcut=128, 1.1 us Tcut=64
with 8 separate instructions and split nb/dash DMAs.
"""

import copy
import operator

import numpy as np

import concourse.bacc as bacc
import concourse.mybir as mybir
import concourse.tile as tile
import concourse.dve_ops as dve_ops
from concourse.bass_utils import run_bass_kernel_spmd
from concourse.dve_spec import Spec, scan, Src0, Src1, AluOp, lower, _has_src1
from concourse.dve_uop import DveOpSpec

B, T = 8192, 1024
N_CORES = 8
ROWS = B // N_CORES  # rows per core
P = 128  # SBUF partitions
NTILES = ROWS // P  # row-tiles per core

f32 = mybir.dt.float32
bf16 = mybir.dt.bfloat16
i8 = mybir.dt.int8
u8 = mybir.dt.uint8
NB_DT = mybir.dt.float8e4

# Candidate device stream lengths and the tail bound (see module docstring).
TCUTS = (48, 64, 128, 256, 512, T)
LOG2_TAIL_BOUND = -30.0

_OP_NAME = "BUZZ_CUMPROD_MUL_REDUCE_SD"


def _op_reference(in0, in1, c0, c1, c2):
    x = (
        np.cumprod(np.asarray(in0, np.float32), axis=-1)
        * np.asarray(in1, np.float32)
    ).astype(np.float32)
    return x, x.reshape(x.shape[0], -1).sum(axis=-1, keepdims=True).astype(np.float32)


_SPEC = Spec(
    body=scan(AluOp.MULTIPLY, Src0) * Src1,
    accum=operator.add,
    reference=_op_reference,
)


def _build_uops():
    """lower() the 2-uop [seed, steady] program, then add per-subdim scan
    reseed: a third uop copying the seed (a 1-cycle non-consuming bubble that
    sets the stage-0 scan flop to 1.0 via the ONE_F32 delay chain) with the
    stage-2 out-flop write DISABLED so the running accum register survives
    the boundary.  steady jumps to it on SUB_DIM_DONE; SRC_TENSOR_DONE stays
    in trigger slot 0 so stream end wins at the final boundary.  HW-verified
    (debug_op.py subdim case)."""
    from concourse.dve_uop import Trigger

    seed, steady = lower(_SPEC, ver="v3")
    steady = copy.deepcopy(steady)
    reseed = copy.deepcopy(seed)
    steady.trigger = (Trigger.SRC_TENSOR_DONE, Trigger.SUB_DIM_DONE, Trigger.NONE)
    steady.next_uop = (0, 2, 0)
    reseed.datapath_config[2].alu_out_enable = 0
    return [seed, steady, reseed]


def _register_op() -> "dve_ops.DveOp":
    for op in dve_ops.OPS:
        if op.name == _OP_NAME:
            return op
    row = max(dve_ops._SUB_OPCODE_FOR_NAME.values()) + 1
    assert row < 0x20, "no free custom-DVE opcode row"
    dve_ops._SUB_OPCODE_FOR_NAME[_OP_NAME] = row
    compiled = DveOpSpec(
        name=_OP_NAME, opcode=row, uops=_build_uops(), rd1_en=_has_src1(_SPEC)
    )
    op = dve_ops.DveOp(
        name=_OP_NAME,
        spec=_SPEC,
        subdim=True,
        uops_sha={"v3": compiled.sha("v3")},
    )
    # pin the hand-patched uop program (compile() would re-lower the Spec)
    dve_ops._COMPILE_CACHE[(_OP_NAME, "v3")] = compiled
    dve_ops.OPS.append(op)
    dve_ops.CUSTOM_DVE_SPECS[_OP_NAME] = _SPEC
    return op


_CACHE = {}


def build_bass(reps: int = 1, tcut: int | None = None):
    tcut = tcut or _CACHE.get("tcut", T)
    op = _register_op()
    nc = bacc.Bacc("TRN2", target_bir_lowering=False, debug=False)
    # nb (fp8 bits) and dash (int8 bits) ride ONE u8 tensor — halves the
    # input-DMA count; on-chip slices are bitcast back to their real dtypes.
    pk = nc.declare_dram_parameter(
        "packed", [P, 2 * NTILES * tcut], u8, isOutput=False
    )
    out = nc.declare_dram_parameter("partials", [P, 1], f32, isOutput=True)

    with tile.TileContext(nc) as tc:
        with (
            tc.tile_pool(name="io", bufs=3) as io_pool,
            tc.tile_pool(name="work", bufs=2) as work_pool,
            tc.tile_pool(name="res", bufs=1) as res_pool,
        ):
            res = res_pool.tile([P, 1], f32)
            # Stock DVE op first: deterministic res init, and the first
            # *custom* DVE decode lands a little after the model-switch
            # table DMA (suspected source of rare first-exec faults).
            nc.vector.memset(res[:], 0.0)
            for rep in range(reps):
                pkt = io_pool.tile(
                    [P, 2 * NTILES, tcut], u8, tag="pk", name=f"pk_{rep}"
                )
                nc.sync.dma_start(pkt[:], pk[:].rearrange("p (s n) -> p s n", n=tcut))
                # ONE instruction per core: 8 sub-tiles with per-subdim scan
                # reseed; the accum register runs through all of them, so
                # accum_out[p] = sum of the 8 rows this partition carries.
                scr = work_pool.tile([P, NTILES, tcut], bf16, tag="scr")
                nc.vector._custom_dve(
                    op,
                    out=scr[:],
                    in0=pkt[:, 0:NTILES, :].bitcast(NB_DT),
                    in1=pkt[:, NTILES : 2 * NTILES, :].bitcast(i8),
                    accum_out=res[:],
                )
            nc.sync.dma_start(out[:], res[:])
    nc.compile()
    return nc


def _pick_tcut(nbq32: np.ndarray) -> int:
    """Smallest Tcut whose worst-row log2(cumprod of the shipped nb values)
    is below LOG2_TAIL_BOUND — i.e. the dropped tail is provably < T*2^-60.
    Falls back to full length when no candidate passes (always exact)."""
    probe = min(max(TCUTS[:-1]), T)
    with np.errstate(divide="ignore"):
        lg = np.log2(nbq32[:, :probe].astype(np.float64))
    cl = np.cumsum(lg, axis=1)
    for tc in TCUTS[:-1]:
        if float(cl[:, tc - 1].max()) < LOG2_TAIL_BOUND:
            return tc
    return T


def _pack(a: np.ndarray, core: int, tcut: int) -> np.ndarray:
    # rows core*ROWS..(core+1)*ROWS-1, cols :tcut  ->  [P, NTILES*tcut]
    # with partition p carrying tile j's row j*P+p at cols j*tcut:(j+1)*tcut.
    c = a[core * ROWS : (core + 1) * ROWS, :tcut]
    return np.ascontiguousarray(
        c.reshape(NTILES, P, tcut).transpose(1, 0, 2).reshape(P, NTILES * tcut)
    )


def make_in_maps(confidences: np.ndarray, accuracies: np.ndarray):
    conf = np.asarray(confidences, dtype=np.float32)
    acc = np.asarray(accuracies, dtype=np.float32)
    nb = np.ones((B, T), np.float32)
    np.subtract(1.0, conf[:, : T - 1], out=nb[:, : T - 1])
    nbb = nb.astype(mybir.dt.np(NB_DT))
    tcut = _pick_tcut(nbb.astype(np.float32))
    _CACHE["tcut"] = tcut
    dash = np.zeros((B, T), np.int8)
    dash[:, : T - 1] = (acc[:, 1:] - acc[:, : T - 1]).astype(np.int8)
    maps = []
    for i in range(N_CORES):
        m = np.concatenate(
            [_pack(nbb.view(np.uint8), i, tcut), _pack(dash.view(np.uint8), i, tcut)],
            axis=1,
        )
        maps.append({"packed": np.ascontiguousarray(m)})
    return maps


def reduce_partials(results, accuracies) -> np.ndarray:
    # device partials + the t=0 boundary term sum_b acc[b, 0]
    total = float(np.sum(np.asarray(accuracies)[:, 0], dtype=np.float64))
    for r in results:
        total += float(np.sum(r["partials"].astype(np.float64)))
    return np.asarray(-(total / B), dtype=np.float32)


def _run_device(confidences: np.ndarray, accuracies: np.ndarray):
    in_maps = make_in_maps(confidences, accuracies)
    tcut = _CACHE["tcut"]
    key = ("nc", tcut)
    if key not in _CACHE:
        _CACHE[key] = build_bass(tcut=tcut)
        _CACHE["nc"] = _CACHE[key]
    return run_bass_kernel_spmd(_CACHE[key], in_maps, list(range(N_CORES))).results


_CHILD_CODE = """
import sys, numpy as np
sys.path.insert(0, sys.argv[1])
import kernel as K
d = np.load(sys.argv[2])
res = K._run_device(d["confidences"], d["accuracies"])
np.savez(sys.argv[3], **{f"p{i}": r["partials"] for i, r in enumerate(res)})
"""


def _run_subprocess(confidences: np.ndarray, accuracies: np.ndarray):
    # Fresh process -> fresh PJRT client; recovers from a transient
    # device-unrecoverable left by a prior NEFF load (NEFF compile is
    # disk-cached, so the retry costs seconds).
    import os
    import subprocess
    import sys
    import tempfile

    here = os.path.dirname(os.path.abspath(__file__))
    with tempfile.TemporaryDirectory() as td:
        in_path = os.path.join(td, "in.npz")
        out_path = os.path.join(td, "out.npz")
        np.savez(in_path, confidences=confidences, accuracies=accuracies)
        subprocess.run(
            [sys.executable, "-c", _CHILD_CODE, here, in_path, out_path],
            check=True,
            timeout=900,
        )
        d = np.load(out_path)
        return [{"partials": d[f"p{i}"]} for i in range(N_CORES)]


def kernel(confidences: np.ndarray, accuracies: np.ndarray) -> np.ndarray:
    import time

    results = None
    try:
        results = _run_device(confidences, accuracies)
    except Exception:
        for attempt in range(3):
            time.sleep(2.0)
            try:
                results = _run_subprocess(confidences, accuracies)
                break
            except Exception:
                if attempt == 2:
                    raise
    return reduce_partials(results, accuracies)


# revision 12
# speedup vs baseline: 4.5418x; 4.5418x over previous
"""BuzzLoss Trainium2 kernel — fused custom-DVE op + adaptive tail truncation.

Math (telescoped form of the reference):
    excl[t] = prod_{s<t} (1 - conf[s])          (exclusive cumprod)
    score_b = sum_t excl[b,t] * da[b,t]
    da[b,0] = acc[b,0];  da[b,t] = acc[b,t] - acc[b,t-1]
    out = -mean_b score_b

With k = t-1 this is  score_b = acc[b,0] + sum_{k>=0} incl[k] * da[k+1]
where incl[k] = prod_{j<=k} nb[j], nb = 1 - conf.  Per 128-row tile the whole
inner sum is ONE custom-DVE instruction:

    Spec(body=scan(MULT, Src0) * Src1, accum=add)
      accum_out[p] = sum_k (prod_{j<=k} Src0[p,j]) * Src1[p,k]

The scan combine uses same-stage CURR_ALU_OUT feedback (no pipeline bubble),
so the op streams at 1 elem/cycle/lane with an fp32 recurrence state — vs the
stock tensor_tensor_scan (half rate) + separate multiply-accumulate pass.

Adaptive tail truncation: incl[k] decays geometrically (each factor <= 1),
so once it provably drops below 2^-25 every remaining term of the row is
|incl*da| <= 2^-25, and the dropped tail is bounded by (T-Tcut)*2^-25 < 3e-5
absolute (rel ~6e-5, a hard all-rows-at-max bound) — 300x below the 2e-2
budget.  The host computes log2-cumsums of the ACTUAL (fp8-rounded) nb it
ships and picks the smallest Tcut in {40, 44, 48, 64, 128, 256, 512, T}
whose worst row across the batch passes the bound; the device streams only
Tcut columns.  This is not a distributional assumption: adversarial inputs
simply select a larger Tcut (up to full length) and stay exact; typical
uniform-confidence data passes at Tcut=40 (max row log2 incl[39] = -26).

Host prep is otherwise dtype/layout only (all reduction work on device):
    nb = fp8_e4m3(1 - conf), dash = int8(acc[:,1:] - acc[:,:-1])
both packed per core into ONE u8 tensor [128, 2*NTILES*Tcut] (partition p
holds tile j's row j*128+p; nb bytes in the first NTILES sub-blocks, dash in
the second) so each core's entire input is a single dense DMA, bitcast back
to fp8/int8 on-chip.
fp8 keeps ~3-bit relative precision on nb (floating format, so the small nb
near conf~1 that drive the decay stay accurate); end-to-end rel err 7e-05.
dash in {-1,0,1} is exact in int8; the DVE converts both dtypes on read.

The t=0 boundary term (= acc[b,0]) and the final mean are host-side, as is
the cross-core reduction (pure data parallel, batch 8192 = 8 x 1024 rows).
The 8 row-tiles per core run as ONE DVE instruction: the in0/in1 APs are
[128, 8, Tcut] and a hand-patched third uop reseeds the scan state at each
SUB_DIM_DONE boundary (keeping the accum register), so per-instruction
dispatch overhead is paid once instead of 8x.  accum_out[p] sums the 8 rows
partition p carries — reduce_partials sums everything anyway.
Steady state at Tcut=40: DVE ~(320+66+7) cyc @ 0.96 GHz ~ 0.41 us/core,
one 80 KiB DMA/core (640 B/partition, above the 512 B line-rate minimum) —
vs 32 us for the fp32 scan+stt baseline, 9.2 us full-length fused-op,
1.7 us Tcut=128, 1.1 us Tcut=64 with 8 instructions and split DMAs,
0.48 us Tcut=48.
"""

import copy
import operator

import numpy as np

import concourse.bacc as bacc
import concourse.mybir as mybir
import concourse.tile as tile
import concourse.dve_ops as dve_ops
from concourse.bass_utils import run_bass_kernel_spmd
from concourse.dve_spec import Spec, scan, Src0, Src1, AluOp, lower, _has_src1
from concourse.dve_uop import DveOpSpec

B, T = 8192, 1024
N_CORES = 8
ROWS = B // N_CORES  # rows per core
P = 128  # SBUF partitions
NTILES = ROWS // P  # row-tiles per core

f32 = mybir.dt.float32
bf16 = mybir.dt.bfloat16
i8 = mybir.dt.int8
u8 = mybir.dt.uint8
NB_DT = mybir.dt.float8e4

# Candidate device stream lengths and the tail bound (see module docstring).
TCUTS = (40, 44, 48, 64, 128, 256, 512, T)
LOG2_TAIL_BOUND = -25.0

_OP_NAME = "BUZZ_CUMPROD_MUL_REDUCE_SD"


def _op_reference(in0, in1, c0, c1, c2):
    x = (
        np.cumprod(np.asarray(in0, np.float32), axis=-1)
        * np.asarray(in1, np.float32)
    ).astype(np.float32)
    return x, x.reshape(x.shape[0], -1).sum(axis=-1, keepdims=True).astype(np.float32)


_SPEC = Spec(
    body=scan(AluOp.MULTIPLY, Src0) * Src1,
    accum=operator.add,
    reference=_op_reference,
)


def _build_uops():
    """lower() the 2-uop [seed, steady] program, then add per-subdim scan
    reseed: a third uop copying the seed (a 1-cycle non-consuming bubble that
    sets the stage-0 scan flop to 1.0 via the ONE_F32 delay chain) with the
    stage-2 out-flop write DISABLED so the running accum register survives
    the boundary.  steady jumps to it on SUB_DIM_DONE; SRC_TENSOR_DONE stays
    in trigger slot 0 so stream end wins at the final boundary.  HW-verified
    (debug_op.py subdim case)."""
    from concourse.dve_uop import Trigger

    seed, steady = lower(_SPEC, ver="v3")
    steady = copy.deepcopy(steady)
    reseed = copy.deepcopy(seed)
    steady.trigger = (Trigger.SRC_TENSOR_DONE, Trigger.SUB_DIM_DONE, Trigger.NONE)
    steady.next_uop = (0, 2, 0)
    reseed.datapath_config[2].alu_out_enable = 0
    return [seed, steady, reseed]


def _register_op() -> "dve_ops.DveOp":
    for op in dve_ops.OPS:
        if op.name == _OP_NAME:
            return op
    row = max(dve_ops._SUB_OPCODE_FOR_NAME.values()) + 1
    assert row < 0x20, "no free custom-DVE opcode row"
    dve_ops._SUB_OPCODE_FOR_NAME[_OP_NAME] = row
    compiled = DveOpSpec(
        name=_OP_NAME, opcode=row, uops=_build_uops(), rd1_en=_has_src1(_SPEC)
    )
    op = dve_ops.DveOp(
        name=_OP_NAME,
        spec=_SPEC,
        subdim=True,
        uops_sha={"v3": compiled.sha("v3")},
    )
    # pin the hand-patched uop program (compile() would re-lower the Spec)
    dve_ops._COMPILE_CACHE[(_OP_NAME, "v3")] = compiled
    dve_ops.OPS.append(op)
    dve_ops.CUSTOM_DVE_SPECS[_OP_NAME] = _SPEC
    return op


_CACHE = {}


def build_bass(reps: int = 1, tcut: int | None = None):
    tcut = tcut or _CACHE.get("tcut", T)
    op = _register_op()
    nc = bacc.Bacc("TRN2", target_bir_lowering=False, debug=False)
    # nb (fp8 bits) and dash (int8 bits) ride ONE u8 tensor — halves the
    # input-DMA count; on-chip slices are bitcast back to their real dtypes.
    pk = nc.declare_dram_parameter(
        "packed", [P, 2 * NTILES * tcut], u8, isOutput=False
    )
    out = nc.declare_dram_parameter("partials", [P, 1], f32, isOutput=True)

    with tile.TileContext(nc) as tc:
        with (
            tc.tile_pool(name="io", bufs=3) as io_pool,
            tc.tile_pool(name="work", bufs=2) as work_pool,
            tc.tile_pool(name="res", bufs=1) as res_pool,
        ):
            res = res_pool.tile([P, 1], f32)
            # Stock DVE op first: deterministic res init, and the first
            # *custom* DVE decode lands a little after the model-switch
            # table DMA (suspected source of rare first-exec faults).
            nc.vector.memset(res[:], 0.0)
            for rep in range(reps):
                pkt = io_pool.tile(
                    [P, 2 * NTILES, tcut], u8, tag="pk", name=f"pk_{rep}"
                )
                nc.sync.dma_start(pkt[:], pk[:].rearrange("p (s n) -> p s n", n=tcut))
                # ONE instruction per core: 8 sub-tiles with per-subdim scan
                # reseed; the accum register runs through all of them, so
                # accum_out[p] = sum of the 8 rows this partition carries.
                scr = work_pool.tile([P, NTILES, tcut], bf16, tag="scr")
                nc.vector._custom_dve(
                    op,
                    out=scr[:],
                    in0=pkt[:, 0:NTILES, :].bitcast(NB_DT),
                    in1=pkt[:, NTILES : 2 * NTILES, :].bitcast(i8),
                    accum_out=res[:],
                )
            nc.sync.dma_start(out[:], res[:])
    nc.compile()
    return nc


def _pick_tcut(nbq32: np.ndarray) -> int:
    """Smallest Tcut whose worst-row log2(cumprod of the shipped nb values)
    is below LOG2_TAIL_BOUND — i.e. the dropped tail is provably < T*2^-60.
    Falls back to full length when no candidate passes (always exact)."""
    probe = min(max(TCUTS[:-1]), T)
    with np.errstate(divide="ignore"):
        lg = np.log2(nbq32[:, :probe].astype(np.float64))
    cl = np.cumsum(lg, axis=1)
    for tc in TCUTS[:-1]:
        if float(cl[:, tc - 1].max()) < LOG2_TAIL_BOUND:
            return tc
    return T


def _pack(a: np.ndarray, core: int, tcut: int) -> np.ndarray:
    # rows core*ROWS..(core+1)*ROWS-1, cols :tcut  ->  [P, NTILES*tcut]
    # with partition p carrying tile j's row j*P+p at cols j*tcut:(j+1)*tcut.
    c = a[core * ROWS : (core + 1) * ROWS, :tcut]
    return np.ascontiguousarray(
        c.reshape(NTILES, P, tcut).transpose(1, 0, 2).reshape(P, NTILES * tcut)
    )


def make_in_maps(confidences: np.ndarray, accuracies: np.ndarray):
    conf = np.asarray(confidences, dtype=np.float32)
    acc = np.asarray(accuracies, dtype=np.float32)
    nb = np.ones((B, T), np.float32)
    np.subtract(1.0, conf[:, : T - 1], out=nb[:, : T - 1])
    nbb = nb.astype(mybir.dt.np(NB_DT))
    tcut = _pick_tcut(nbb.astype(np.float32))
    _CACHE["tcut"] = tcut
    dash = np.zeros((B, T), np.int8)
    dash[:, : T - 1] = (acc[:, 1:] - acc[:, : T - 1]).astype(np.int8)
    maps = []
    for i in range(N_CORES):
        m = np.concatenate(
            [_pack(nbb.view(np.uint8), i, tcut), _pack(dash.view(np.uint8), i, tcut)],
            axis=1,
        )
        maps.append({"packed": np.ascontiguousarray(m)})
    return maps


def reduce_partials(results, accuracies) -> np.ndarray:
    # device partials + the t=0 boundary term sum_b acc[b, 0]
    total = float(np.sum(np.asarray(accuracies)[:, 0], dtype=np.float64))
    for r in results:
        total += float(np.sum(r["partials"].astype(np.float64)))
    return np.asarray(-(total / B), dtype=np.float32)


def _run_device(confidences: np.ndarray, accuracies: np.ndarray):
    in_maps = make_in_maps(confidences, accuracies)
    tcut = _CACHE["tcut"]
    key = ("nc", tcut)
    if key not in _CACHE:
        _CACHE[key] = build_bass(tcut=tcut)
        _CACHE["nc"] = _CACHE[key]
    return run_bass_kernel_spmd(_CACHE[key], in_maps, list(range(N_CORES))).results


_CHILD_CODE = """
import sys, numpy as np
sys.path.insert(0, sys.argv[1])
import kernel as K
d = np.load(sys.argv[2])
res = K._run_device(d["confidences"], d["accuracies"])
np.savez(sys.argv[3], **{f"p{i}": r["partials"] for i, r in enumerate(res)})
"""


def _run_subprocess(confidences: np.ndarray, accuracies: np.ndarray):
    # Fresh process -> fresh PJRT client; recovers from a transient
    # device-unrecoverable left by a prior NEFF load (NEFF compile is
    # disk-cached, so the retry costs seconds).
    import os
    import subprocess
    import sys
    import tempfile

    here = os.path.dirname(os.path.abspath(__file__))
    with tempfile.TemporaryDirectory() as td:
        in_path = os.path.join(td, "in.npz")
        out_path = os.path.join(td, "out.npz")
        np.savez(in_path, confidences=confidences, accuracies=accuracies)
        subprocess.run(
            [sys.executable, "-c", _CHILD_CODE, here, in_path, out_path],
            check=True,
            timeout=900,
        )
        d = np.load(out_path)
        return [{"partials": d[f"p{i}"]} for i in range(N_CORES)]


def kernel(confidences: np.ndarray, accuracies: np.ndarray) -> np.ndarray:
    import time

    results = None
    try:
        results = _run_device(confidences, accuracies)
    except Exception:
        for attempt in range(3):
            time.sleep(2.0)
            try:
                results = _run_subprocess(confidences, accuracies)
                break
            except Exception:
                if attempt == 2:
                    raise
    return reduce_partials(results, accuracies)
